# revision 9
# baseline (speedup 1.0000x reference)
"""Trainium2 Bass kernel for gnn_message_passing (gather + matmul).

Reference computation:
    out[b, m, p] = sum_{c,k} W[m, c*KS+k] * x[b, c, idx[p, k]]
with B=32, C=32, P=4096 pixels, KS=9 neighbors, K=64 output channels.

Strategy (8 NeuronCores, pixel-parallel with a replicated token table):
  The gather is the expensive part: SWDGE descriptor generation on the
  GPSIMD Q7 costs ~10ns per gathered token, so we minimize token COUNT by
  maximizing token SIZE.  idx is shared by every (batch, channel), so a
  token for pixel q packs all 32*32 = 1024 values x[:, :, q] (2KB bf16).

  Per core:
   - DMA the FULL x (1024, 4096) f32 in 8 slabs of 128 (b,c) rows; PE-
     transpose 128x128 blocks (f32 -> PSUM), DVE copy-cast to bf16, and
     DMA out to a DRAM token table tbl[q, bc] (4096 x 1024 bf16, 2KB rows).
   - dma_gather (HBM source, transpose=True, elem=1024) with int16 index
     lists for THIS core's 512 pixels: 9 calls x 512 idxs.  Gathered
     G[p128, k, f, i] = x[bc=f*128+p128, idx[pix_i, k]] - the matmul rhs
     with the contraction (b%4, c) on partitions, batch-group f on free.
   - Matmuls with block-diagonal weights: lhsT BD[bp,k] (128x128 bf16)
     maps rhs partitions (b', c) -> out partitions (j, m) for batches
     4f+2bp+j, accumulating the 9 k's in PSUM (f32).  Same BD reused for
     every batch group f.
   - PSUM -> SBUF (DVE) -> DRAM out (2048, 512) f32 = (f, bp, j, m) x pix.
"""

import os

import numpy as np
import ml_dtypes

import concourse.bass as bass
import concourse.mybir as mybir
import concourse.tile as tile
from concourse import bacc
from concourse.bass_utils import run_bass_kernel_spmd

B, C, H, W_IMG = 32, 32, 64, 64
P = H * W_IMG          # 4096 pixels
KS = 9                 # neighbors per pixel
K = 64                 # output channels
NCORES = 8
PPC = P // NCORES      # 512 pixels per core
NBC = B * C            # 1024 = full (b, c) dim
NSLAB = NBC // 128     # 8 slabs
NF = NSLAB             # 8 batch groups of 4 on the gather free dim
NQUEUES = 4            # SWDGE queues for gather desc-gen overlap

_cache = {}


def _build():
    nc = bacc.Bacc("TRN2", target_bir_lowering=False, debug=False,
                   num_devices=NCORES, num_swdge_queues=NQUEUES)

    x_ext = nc.dram_tensor("x", [NBC, P], mybir.dt.float32,
                           kind="ExternalInput")
    wbd_ext = nc.dram_tensor("wbd", [128, 2 * KS * 128], mybir.dt.bfloat16,
                             kind="ExternalInput")
    idx_ext = nc.dram_tensor("idx16", [128, KS * PPC // 16], mybir.dt.int16,
                             kind="ExternalInput")
    out_ext = nc.dram_tensor("out", [B * K, PPC],
                             mybir.dt.float32, kind="ExternalOutput")
    tbl = nc.dram_tensor("tbl", [P, NBC], mybir.dt.bfloat16)

    with tile.TileContext(nc) as tc:
        with (
            tc.tile_pool(name="persist", bufs=1) as pp,
            tc.tile_pool(name="slab", bufs=2) as slp,
            tc.tile_pool(name="stage", bufs=3) as sp,
            tc.tile_pool(name="pstr", bufs=2, space="PSUM") as ptr,
            tc.tile_pool(name="psmm", bufs=4, space="PSUM") as pmm,
        ):
            idx_t = pp.tile([128, KS * PPC // 16], mybir.dt.int16, tag="idx")
            bd_t = pp.tile([128, 2 * KS, 128], mybir.dt.bfloat16, tag="bd")
            ident = pp.tile([128, 128], mybir.dt.float32, tag="ident")
            G = pp.tile([128, KS, NF, PPC], mybir.dt.bfloat16, tag="G")

            nc.sync.dma_start(idx_t[:], idx_ext[:, :])
            nc.sync.dma_start(bd_t[:], wbd_ext[:, :].rearrange(
                "p (a b) -> p a b", b=128))

            from concourse.masks import make_identity
            make_identity(nc, ident[:])

            # token table: tbl[q, bc] = x[bc, q] (bf16)
            for s in range(NSLAB):
                Xs = slp.tile([128, P], mybir.dt.float32, tag="Xs")
                nc.sync.dma_start(Xs[:], x_ext[s * 128:(s + 1) * 128, :])
                Ts = slp.tile([128, 32, 128], mybir.dt.bfloat16, tag="Ts")
                for g in range(8):
                    pt = ptr.tile([128, 4, 128], mybir.dt.float32,
                                  tag="pt")
                    for r4 in range(4):
                        r = g * 4 + r4
                        nc.tensor.transpose(
                            pt[:, r4, :], Xs[:, r * 128:(r + 1) * 128],
                            ident[:])
                    nc.vector.tensor_copy(out=Ts[:, g * 4:(g + 1) * 4, :],
                                          in_=pt[:])
                nc.sync.dma_start(
                    tbl[:, s * 128:(s + 1) * 128].rearrange(
                        "(r p) e -> p r e", p=128),
                    Ts[:])

            # gather: G[p, k, f, i] = tbl[idx[pix_i, k], f*128+p]
            for k in range(KS):
                nc.gpsimd.dma_gather(
                    G[:, k, :, :],
                    tbl[:, :],
                    idx_t[:, k * (PPC // 16):(k + 1) * (PPC // 16)],
                    PPC,        # num_idxs
                    PPC,        # num_idxs_reg (all valid)
                    NBC,        # elem_size (bf16 elements = 2KB)
                    transpose=True,
                    queue_num=k % NQUEUES,
                )

            # matmuls: batch group f, pair bp -> batches 4f+2bp+{0,1}
            for f in range(NF):
                for bp in range(2):
                    ps = pmm.tile([128, PPC], mybir.dt.float32, tag="ps_mm")
                    for k in range(KS):
                        nc.tensor.matmul(
                            ps[:],
                            bd_t[:, bp * KS + k, :],
                            G[:, k, f, :],
                            start=(k == 0),
                            stop=(k == KS - 1),
                        )
                    st = sp.tile([128, PPC], mybir.dt.float32, tag="st")
                    nc.vector.tensor_copy(out=st[:], in_=ps[:])
                    row = (f * 2 + bp) * 128
                    nc.sync.dma_start(out_ext[row:row + 128, :], st[:])

    nc.compile()
    return nc


def _get_nc():
    if "nc" not in _cache:
        _cache["nc"] = _build()
    return _cache["nc"]


def _prep_idx16(idx: np.ndarray) -> list:
    """idx (1,64,64,9) int32 -> per-core (128, KS*PPC//16) int16 lists.

    Core i handles pixels [PPC*i, PPC*(i+1)).  Chunk k holds idx[p, k] for
    those pixels, wrapped: element j at partition j%16, col j//16
    (replicated to the 8 16-partition groups)."""
    lst = idx.reshape(P, KS).astype(np.int16)
    outs = []
    for i in range(NCORES):
        o = np.zeros((128, KS * (PPC // 16)), dtype=np.int16)
        for k in range(KS):
            w = lst[PPC * i:PPC * (i + 1), k].reshape(PPC // 16, 16).T
            o[:, k * (PPC // 16):(k + 1) * (PPC // 16)] = np.tile(w, (8, 1))
        outs.append(o)
    return outs


def _prep_wbd(weights: np.ndarray) -> np.ndarray:
    """weights (64, 288) f32 -> block-diag lhsT set (128, 2*KS*128) bf16.

    BD[bp, k][32*b' + c, 64*j + m] = W[m, c*KS+k] if b' == 2*bp+j else 0,
    for b' in 0..4 (batch-within-group); reused for every group f."""
    bd = np.zeros((2, KS, 128, 128), dtype=np.float32)
    for k in range(KS):
        wk = weights[:, k::KS]  # (64, 32) = W[m, c*KS+k]
        for bp in range(2):
            for j in range(2):
                bprime = 2 * bp + j
                bd[bp, k, 32 * bprime:32 * bprime + 32, 64 * j:64 * j + 64] = \
                    wk.T
    return bd.reshape(2 * KS, 128, 128).transpose(1, 0, 2).reshape(
        128, 2 * KS * 128).astype(ml_dtypes.bfloat16)


def prep_in_maps(x: np.ndarray, weights: np.ndarray, idx: np.ndarray):
    idx16s = _prep_idx16(np.asarray(idx))
    wbd = _prep_wbd(np.asarray(weights, dtype=np.float32))
    xf = np.ascontiguousarray(
        np.asarray(x, dtype=np.float32).reshape(NBC, P))
    return [{"x": xf, "wbd": wbd, "idx16": idx16s[i]} for i in range(NCORES)]


def assemble_out(results) -> np.ndarray:
    out = np.empty((B, K, P), dtype=np.float32)
    for i in range(NCORES):
        r = results[i]["out"].reshape(NF, 2, 2, K, PPC)  # (f, bp, j, m, p)
        for f in range(NF):
            for bp in range(2):
                for j in range(2):
                    out[4 * f + 2 * bp + j, :, PPC * i:PPC * (i + 1)] = \
                        r[f, bp, j]
    return out.reshape(B, K, H, W_IMG)


last_results = None


def kernel(x, weights, idx):
    global last_results
    nc = _get_nc()
    in_maps = prep_in_maps(x, weights, idx)
    trace = bool(int(os.environ.get("KERNEL_TRACE", "0")))
    res = run_bass_kernel_spmd(nc, in_maps, core_ids=list(range(NCORES)),
                               trace=trace)
    last_results = res
    return assemble_out(res.results)
